# revision 1
# baseline (speedup 1.0000x reference)
"""ChatGLM2 attention block on 8 Trainium2 NeuronCores.

Tensor-parallel across heads: core c computes KV-group c//4 and Q-heads
4*(c%4)..4*(c%4)+3, plus the matching row-slice of the output projection.
Partial o_proj outputs are summed on the host (row-parallel unshard).
All matmuls run in float32r (reduced-precision fp32, ~1e-4 rel err).
"""

import numpy as np

import concourse.bass as bass
import concourse.mybir as mybir
from concourse import bacc
from concourse.tile import TileContext
from concourse.bass_utils import run_bass_kernel_spmd

# problem shape (hardcoded per contract)
B, S, NS, H, G, D = 1, 2048, 4096, 32, 2, 128
HG = H // G          # 16 heads per group
NH = 4               # q heads per core
NCORES = 8
NSC = NS // 128      # 32 contraction chunks
SC = S // 128        # 16 sequence chunks
NQT = 4              # q tiles of 512
QW = 512             # q tile width
SCALE = 1.0 / float(np.sqrt(D))

f32 = mybir.dt.float32
f32r = mybir.dt.float32r

_CACHE = {}


def _build_program():
    nc = bacc.Bacc("TRN2", target_bir_lowering=False, debug=False,
                   num_devices=NCORES)

    xa = nc.declare_dram_parameter("xa", [SC, 128, NSC, 128], f32r, isOutput=False)
    w = nc.declare_dram_parameter("w", [128, NSC, 768], f32r, isOutput=False)
    wo = nc.declare_dram_parameter("wo", [32, 128, 4, 128], f32r, isOutput=False)
    cosn = nc.declare_dram_parameter("cosn", [128, SC, 64], f32, isOutput=False)
    sinn = nc.declare_dram_parameter("sinn", [128, SC, 64], f32, isOutput=False)
    maskt = nc.declare_dram_parameter("maskt", [128, 4, QW], f32, isOutput=False)
    bias_b = nc.declare_dram_parameter("bias_b", [128, 768], f32, isOutput=False)
    ident_i = nc.declare_dram_parameter("ident_i", [128, 128], f32r, isOutput=False)
    ones_c = nc.declare_dram_parameter("ones_c", [128, 1], f32r, isOutput=False)
    ones_r = nc.declare_dram_parameter("ones_r", [1, 128], f32r, isOutput=False)

    outp = nc.declare_dram_parameter("outp", [32, 128, S], f32, isOutput=True)
    kc = nc.declare_dram_parameter("kc", [128, S], f32r, isOutput=True)
    vc = nc.declare_dram_parameter("vc", [128, SC, 128], f32r, isOutput=True)

    Copy = mybir.ActivationFunctionType.Copy
    Exp = mybir.ActivationFunctionType.Exp

    with TileContext(nc) as tc:
        with (
            tc.tile_pool(name="persist", bufs=1) as persist,
            tc.tile_pool(name="accA", bufs=2, space="PSUM") as accA,
            tc.tile_pool(name="accB", bufs=2, space="PSUM") as accB,
            tc.tile_pool(name="poP", bufs=2, space="PSUM") as poP,
            tc.tile_pool(name="pdP", bufs=2, space="PSUM") as pdP,
        ):
            qt = persist.tile([128, NH, S], f32r, tag="qt")
            kt = persist.tile([128, S], f32r, tag="kt")
            vnat = persist.tile([128, SC, 128], f32r, tag="vnat")
            cos_sb = persist.tile([128, SC, 64], f32, tag="cos")
            sin_sb = persist.tile([128, SC, 64], f32, tag="sin")
            bias_sb = persist.tile([128, 768], f32, tag="bias")
            ident = persist.tile([128, 128], f32r, tag="ident")
            onesc = persist.tile([128, 1], f32r, tag="onesc")
            onesr = persist.tile([1, 128], f32r, tag="onesr")
            nc.sync.dma_start(out=cos_sb[:], in_=cosn[:])
            nc.sync.dma_start(out=sin_sb[:], in_=sinn[:])
            nc.sync.dma_start(out=bias_sb[:], in_=bias_b[:])
            nc.sync.dma_start(out=ident[:], in_=ident_i[:])
            nc.sync.dma_start(out=onesc[:], in_=ones_c[:])
            nc.sync.dma_start(out=onesr[:], in_=ones_r[:])

            # ---- Phase A: QKV projection + bias + RoPE + Q/K transpose ----
            with (
                tc.tile_pool(name="wpool", bufs=1) as wpool,
                tc.tile_pool(name="xsp", bufs=2) as xsp,
                tc.tile_pool(name="stage", bufs=2) as stage,
            ):
                w_sb = wpool.tile([128, NSC, 768], f32r, tag="w")
                nc.sync.dma_start(out=w_sb[:], in_=w[:])
                for sc in range(SC):
                    xs = xsp.tile([128, NSC, 128], f32r, tag="xs")
                    nc.sync.dma_start(out=xs[:], in_=xa[sc])
                    psA = accA.tile([128, 512], f32, tag="accA")
                    psB = accB.tile([128, 256], f32, tag="accB")
                    for ns in range(NSC):
                        nc.tensor.matmul(psA[:], xs[:, ns, :], w_sb[:, ns, 0:512],
                                         start=(ns == 0), stop=(ns == NSC - 1))
                        nc.tensor.matmul(psB[:], xs[:, ns, :], w_sb[:, ns, 512:768],
                                         start=(ns == 0), stop=(ns == NSC - 1))
                    # bias add -> natural fp32 staging (rope region: cols 0:640)
                    nat = stage.tile([128, 640], f32, tag="nat")
                    nc.vector.tensor_add(nat[:, 0:512], psA[:], bias_sb[:, 0:512])
                    nc.vector.tensor_add(nat[:, 512:640], psB[:, 0:128],
                                         bias_sb[:, 512:640])
                    # v slice: bias add straight to f32r natural V
                    nc.vector.tensor_add(vnat[:, sc, :], psB[:, 128:256],
                                         bias_sb[:, 640:768])
                    # rope on [k,q0..q3]: nat viewed [128, 5, 2, 64]
                    natv = nat[:].rearrange("p (h t d) -> p h t d", h=5, t=2)
                    even = natv[:, :, 0, :]
                    odd = natv[:, :, 1, :]
                    cb = cos_sb[:, sc:sc + 1, :].broadcast_to([128, 5, 64])
                    sb_ = sin_sb[:, sc:sc + 1, :].broadcast_to([128, 5, 64])
                    rt = stage.tile([128, 5, 2, 64], f32r, tag="rt")
                    t1 = stage.tile([128, 5, 64], f32, tag="t1")
                    t2 = stage.tile([128, 5, 64], f32, tag="t2")
                    nc.vector.tensor_mul(t1[:], even, cb)
                    nc.vector.tensor_mul(t2[:], odd, sb_)
                    nc.vector.tensor_sub(rt[:, :, 0, :], t1[:], t2[:])
                    t3 = stage.tile([128, 5, 64], f32, tag="t1")
                    t4 = stage.tile([128, 5, 64], f32, tag="t2")
                    nc.vector.tensor_mul(t3[:], even, sb_)
                    nc.vector.tensor_mul(t4[:], odd, cb)
                    nc.vector.tensor_add(rt[:, :, 1, :], t3[:], t4[:])
                    # transpose the 5 rope'd heads into [D, S] layout
                    for r in range(5):
                        pt = accB.tile([128, 128], f32r, tag="accB")
                        nc.tensor.transpose(
                            pt[:], rt[:, r, :, :].rearrange("p t d -> p (t d)"),
                            ident[:])
                        dst = kt[:, sc * 128:(sc + 1) * 128] if r == 0 else \
                            qt[:, r - 1, sc * 128:(sc + 1) * 128]
                        nc.scalar.activation(dst, pt[:], Copy)

            # kv cache outputs
            nc.sync.dma_start(out=kc[:], in_=kt[:])
            nc.sync.dma_start(out=vc[:], in_=vnat[:])

            # ---- Phase B: attention ----
            with tc.tile_pool(name="aout", bufs=1) as aoutp:
                aout = aoutp.tile([128, NH, S], f32r, tag="aout")
                with tc.tile_pool(name="battn", bufs=2) as battn:
                    mask_sb = battn.tile([128, 4, QW], f32, tag="maskt")
                    nc.sync.dma_start(out=mask_sb[:], in_=maskt[:])
                    for h in range(NH):
                        for t in range(NQT):
                            nkc = 4 * (t + 1)
                            po = poP.tile([128, QW], f32, tag="po")
                            pd = pdP.tile([1, QW], f32, tag="pd")
                            for j in range(nkc):
                                pscore = accA.tile([128, QW], f32, tag="accA")
                                nc.tensor.matmul(
                                    pscore[:], kt[:, j * 128:(j + 1) * 128],
                                    qt[:, h, t * QW:(t + 1) * QW],
                                    start=True, stop=True)
                                if j >= 4 * t:
                                    c = j - 4 * t
                                    masked = battn.tile([128, QW], f32, tag="msk")
                                    nc.vector.tensor_add(masked[:], pscore[:],
                                                         mask_sb[:, c, :])
                                    src = masked
                                else:
                                    src = pscore
                                pexp = battn.tile([128, QW], f32r, tag="pexp")
                                nc.scalar.activation(pexp[:], src[:], Exp,
                                                     scale=SCALE)
                                nc.tensor.matmul(po[:], vnat[:, j, :], pexp[:],
                                                 start=(j == 0),
                                                 stop=(j == nkc - 1))
                                nc.tensor.matmul(pd[:], onesc[:], pexp[:],
                                                 start=(j == 0),
                                                 stop=(j == nkc - 1))
                            recip = battn.tile([1, QW], f32r, tag="recip")
                            with nc.allow_low_precision(reason="f32r storage"):
                                nc.vector.reciprocal(recip[:], pd[:])
                            pb = accA.tile([128, QW], f32, tag="accA")
                            nc.tensor.matmul(pb[:], onesr[:], recip[:],
                                             start=True, stop=True)
                            pbs = battn.tile([128, QW], f32, tag="pbs")
                            nc.scalar.activation(pbs[:], pb[:], Copy)
                            nc.vector.tensor_mul(
                                aout[:, h, t * QW:(t + 1) * QW], po[:], pbs[:])

                # ---- Phase C: output projection (partial) ----
                with tc.tile_pool(name="oproj", bufs=3) as oproj:
                    for n in range(32):
                        wo_t = oproj.tile([128, 4, 128], f32r, tag="wo")
                        nc.sync.dma_start(out=wo_t[:], in_=wo[n])
                        for st in range(NQT):
                            pso = accA.tile([128, QW], f32, tag="accA")
                            for hd in range(4):
                                nc.tensor.matmul(
                                    pso[:], wo_t[:, hd, :],
                                    aout[:, hd, st * QW:(st + 1) * QW],
                                    start=(hd == 0), stop=(hd == 3))
                            ob = oproj.tile([128, QW], f32, tag="ob")
                            nc.vector.tensor_copy(ob[:], pso[:])
                            nc.sync.dma_start(
                                out=outp[n, :, st * QW:(st + 1) * QW], in_=ob[:])

    nc.compile()
    return nc


# even/odd de-interleave permutation of the head dim
_PERM = np.concatenate([np.arange(0, D, 2), np.arange(1, D, 2)])


def _prep_inputs(x, rotary_emb, attention_mask, Wqkv, bqkv, Wo):
    x = np.asarray(x, dtype=np.float32)
    rotary_emb = np.asarray(rotary_emb, dtype=np.float32)
    attention_mask = np.asarray(attention_mask, dtype=np.float32)
    Wqkv = np.asarray(Wqkv, dtype=np.float32)
    bqkv = np.asarray(bqkv, dtype=np.float32)
    Wo = np.asarray(Wo, dtype=np.float32)

    # x [1,S,NS] -> xa [SC, 128(p=ns%128), NSC, 128(s%128)]
    xa = np.ascontiguousarray(
        x[0].reshape(SC, 128, NSC, 128).transpose(0, 3, 2, 1))

    # rope tables in natural layout [s%128, sc, i]
    cosn = np.ascontiguousarray(
        rotary_emb[:, 0, 0, :, 0].reshape(SC, 128, 64).transpose(1, 0, 2))
    sinn = np.ascontiguousarray(
        rotary_emb[:, 0, 0, :, 1].reshape(SC, 128, 64).transpose(1, 0, 2))

    # diagonal-relative mask tiles [128(k), 4(c), 512(q)], pre-scaled by 1/SCALE
    m = attention_mask[0]
    maskt = np.ascontiguousarray(
        np.stack([m[0:QW, 128 * c:128 * (c + 1)].T for c in range(4)], axis=1)
        * (1.0 / SCALE)).astype(np.float32)

    ident = np.eye(128, dtype=np.float32)
    ones_c = np.ones((128, 1), dtype=np.float32)
    ones_r = np.ones((1, 128), dtype=np.float32)

    in_maps = []
    for core in range(NCORES):
        g = core // 4
        hg0 = 4 * (core % 4)
        # row indices into Wqkv, column order [k, q0..q3, v]
        rows = [NS + g * D + _PERM]
        for hh in range(NH):
            rows.append((g * HG + hg0 + hh) * D + _PERM)
        rows.append(NS + G * D + g * D + np.arange(D))
        rows = np.concatenate(rows)                      # [768]
        w_core = Wqkv[rows, :]                           # [768, NS]
        # w [128(p=ns%128), NSC, 768]
        w_arr = np.ascontiguousarray(
            w_core.T.reshape(NSC, 128, 768).transpose(1, 0, 2))
        bias_row = bqkv[rows]                            # [768]
        bias_bcast = np.ascontiguousarray(
            np.broadcast_to(bias_row[None, :], (128, 768))).astype(np.float32)
        # wo [32(n), 128(p=hd%128), 4(hd chunk), 128(nn)]
        cols = np.arange((g * HG + hg0) * D, (g * HG + hg0 + NH) * D)
        wo_arr = np.ascontiguousarray(
            Wo[:, cols].reshape(32, 128, 4, 128).transpose(0, 3, 2, 1))
        in_maps.append({
            "xa": xa, "w": w_arr, "wo": wo_arr, "cosn": cosn, "sinn": sinn,
            "maskt": maskt, "bias_b": bias_bcast, "ident_i": ident,
            "ones_c": ones_c, "ones_r": ones_r,
        })
    return in_maps


def kernel(x, rotary_emb, attention_mask, Wqkv, bqkv, Wo, _trace=False):
    if "nc" not in _CACHE:
        _CACHE["nc"] = _build_program()
    nc = _CACHE["nc"]
    in_maps = _prep_inputs(x, rotary_emb, attention_mask, Wqkv, bqkv, Wo)
    res = run_bass_kernel_spmd(nc, in_maps, list(range(NCORES)), trace=_trace)
    _CACHE["last_result"] = res

    # unshard: sum o_proj partials, transpose back to [1, S, NS]
    out_t = np.zeros((NS, S), dtype=np.float32)
    for core in range(NCORES):
        out_t += res.results[core]["outp"].reshape(NS, S)
    out = np.ascontiguousarray(out_t.T)[None]            # [1, S, NS]

    inv_perm = np.argsort(_PERM)
    k_cache = np.empty((B, S, G, 1, D), dtype=np.float32)
    v_cache = np.empty((B, S, G, 1, D), dtype=np.float32)
    for g, core in ((0, 0), (1, 4)):
        kt_rows = res.results[core]["kc"]                # [128(perm'd d), S]
        k_cache[0, :, g, 0, :] = kt_rows[inv_perm, :].T
        vn = res.results[core]["vc"]                     # [128(s%128), SC, 128]
        v_cache[0, :, g, 0, :] = vn.transpose(1, 0, 2).reshape(S, D)
    return out, k_cache, v_cache


# revision 3
# speedup vs baseline: 1.0712x; 1.0712x over previous
"""ChatGLM2 attention block on 8 Trainium2 NeuronCores.

Tensor-parallel across heads: core c computes KV-group c//4 and Q-heads
4*(c%4)..4*(c%4)+3, plus the matching row-slice of the output projection.
Partial o_proj outputs are summed on the host (row-parallel unshard).
All matmuls run in float32r (reduced-precision fp32, ~1e-4 rel err).
"""

import numpy as np

import concourse.bass as bass
import concourse.mybir as mybir
from concourse import bacc
from concourse.tile import TileContext
from concourse.bass_utils import run_bass_kernel_spmd

# problem shape (hardcoded per contract)
B, S, NS, H, G, D = 1, 2048, 4096, 32, 2, 128
HG = H // G          # 16 heads per group
NH = 4               # q heads per core
NCORES = 8
NSC = NS // 128      # 32 contraction chunks
SC = S // 128        # 16 sequence chunks
NQT = 4              # q tiles of 512
QW = 512             # q tile width
SCALE = 1.0 / float(np.sqrt(D))
LOOKAHEAD = 4        # attention pscore pipeline depth

f32 = mybir.dt.float32
f32r = mybir.dt.float32r

_CACHE = {}


def _build_program():
    nc = bacc.Bacc("TRN2", target_bir_lowering=False, debug=False,
                   num_devices=NCORES)

    xa = nc.declare_dram_parameter("xa", [SC, 128, NSC, 128], f32r, isOutput=False)
    w = nc.declare_dram_parameter("w", [128, NSC, 768], f32r, isOutput=False)
    wo = nc.declare_dram_parameter("wo", [32, 128, 4, 128], f32r, isOutput=False)
    cosn = nc.declare_dram_parameter("cosn", [128, SC, 64], f32, isOutput=False)
    sinn = nc.declare_dram_parameter("sinn", [128, SC, 64], f32, isOutput=False)
    maskt = nc.declare_dram_parameter("maskt", [128, 4, QW], f32, isOutput=False)
    bias_b = nc.declare_dram_parameter("bias_b", [128, 768], f32, isOutput=False)
    ident_i = nc.declare_dram_parameter("ident_i", [128, 128], f32r, isOutput=False)
    ones_c = nc.declare_dram_parameter("ones_c", [128, 1], f32r, isOutput=False)

    outp = nc.declare_dram_parameter("outp", [32, 128, S], f32, isOutput=True)
    kc = nc.declare_dram_parameter("kc", [128, S], f32r, isOutput=True)
    vc = nc.declare_dram_parameter("vc", [128, SC, 128], f32r, isOutput=True)

    Copy = mybir.ActivationFunctionType.Copy
    Exp = mybir.ActivationFunctionType.Exp

    with TileContext(nc) as tc:
        with (
            tc.tile_pool(name="persist", bufs=1) as persist,
            tc.tile_pool(name="ps", bufs=1, space="PSUM") as ps,
        ):
            def big(name="big"):
                return ps.tile([128, 512], f32, tag="big5", bufs=5, name=name)

            qt = persist.tile([128, NH, S], f32r, tag="qt")
            kt = persist.tile([128, S], f32r, tag="kt")
            vnat = persist.tile([128, SC, 128], f32r, tag="vnat")
            cos_sb = persist.tile([128, SC, 64], f32, tag="cos")
            sin_sb = persist.tile([128, SC, 64], f32, tag="sin")
            bias_sb = persist.tile([128, 768], f32, tag="bias")
            ident = persist.tile([128, 128], f32r, tag="ident")
            onesc = persist.tile([128, 1], f32r, tag="onesc")
            nc.sync.dma_start(out=cos_sb[:], in_=cosn[:])
            nc.sync.dma_start(out=sin_sb[:], in_=sinn[:])
            nc.sync.dma_start(out=bias_sb[:], in_=bias_b[:])
            nc.sync.dma_start(out=ident[:], in_=ident_i[:])
            nc.sync.dma_start(out=onesc[:], in_=ones_c[:])

            # ---- Phase A: QKV projection + bias + RoPE + Q/K transpose ----
            with (
                tc.tile_pool(name="wpool", bufs=1) as wpool,
                tc.tile_pool(name="xsp", bufs=2) as xsp,
                tc.tile_pool(name="stage", bufs=2) as stage,
            ):
                w_parts = []
                for i in range(4):
                    wp = wpool.tile([128, 8, 768], f32r, tag=f"w{i}")
                    nc.sync.dma_start(out=wp[:], in_=w[:, i * 8:(i + 1) * 8, :])
                    w_parts.append(wp)

                def emit_proj(sc):
                    xs = xsp.tile([128, NSC, 128], f32r, tag="xs")
                    nc.sync.dma_start(out=xs[:], in_=xa[sc])
                    psA = big()
                    psB = ps.tile([128, 256], f32, tag="aux2", bufs=2)
                    for ns in range(NSC):
                        wv = w_parts[ns // 8][:, ns % 8, :]
                        nc.tensor.matmul(psA[:], xs[:, ns, :], wv[:, 0:512],
                                         start=(ns == 0), stop=(ns == NSC - 1))
                        nc.tensor.matmul(psB[:], xs[:, ns, :], wv[:, 512:768],
                                         start=(ns == 0), stop=(ns == NSC - 1))
                    return psA, psB

                def emit_tail(sc, psA, psB):
                    # bias add -> natural fp32 staging (rope cols 0:640)
                    nat = stage.tile([128, 640], f32, tag="nat")
                    nc.vector.tensor_add(nat[:, 0:512], psA[:], bias_sb[:, 0:512])
                    nc.vector.tensor_add(nat[:, 512:640], psB[:, 0:128],
                                         bias_sb[:, 512:640])
                    nc.vector.tensor_add(vnat[:, sc, :], psB[:, 128:256],
                                         bias_sb[:, 640:768])
                    natv = nat[:].rearrange("p (h t d) -> p h t d", h=5, t=2)
                    even = natv[:, :, 0, :]
                    odd = natv[:, :, 1, :]
                    cb = cos_sb[:, sc:sc + 1, :].broadcast_to([128, 5, 64])
                    sb_ = sin_sb[:, sc:sc + 1, :].broadcast_to([128, 5, 64])
                    rt = stage.tile([128, 5, 2, 64], f32r, tag="rt")
                    t1 = stage.tile([128, 5, 64], f32, tag="t1")
                    t2 = stage.tile([128, 5, 64], f32, tag="t2")
                    nc.vector.tensor_mul(t1[:], even, cb)
                    nc.vector.tensor_mul(t2[:], odd, sb_)
                    nc.vector.tensor_sub(rt[:, :, 0, :], t1[:], t2[:])
                    t3 = stage.tile([128, 5, 64], f32, tag="t1")
                    t4 = stage.tile([128, 5, 64], f32, tag="t2")
                    nc.vector.tensor_mul(t3[:], even, sb_)
                    nc.vector.tensor_mul(t4[:], odd, cb)
                    nc.vector.tensor_add(rt[:, :, 1, :], t3[:], t4[:])
                    for r in range(5):
                        pt = ps.tile([128, 128], f32r, tag="big5", bufs=5)
                        nc.tensor.transpose(
                            pt[:], rt[:, r, :, :].rearrange("p t d -> p (t d)"),
                            ident[:])
                        dst = kt[:, sc * 128:(sc + 1) * 128] if r == 0 else \
                            qt[:, r - 1, sc * 128:(sc + 1) * 128]
                        nc.scalar.activation(dst, pt[:], Copy)

                # software pipeline: proj(sc) ahead of tail(sc-1)
                prev = None
                for sc in range(SC):
                    cur = emit_proj(sc)
                    if prev is not None:
                        emit_tail(sc - 1, *prev)
                    prev = cur
                emit_tail(SC - 1, *prev)

            # kv cache outputs
            nc.sync.dma_start(out=kc[:], in_=kt[:])
            nc.sync.dma_start(out=vc[:], in_=vnat[:])

            # ---- Phase B: attention ----
            with tc.tile_pool(name="aout", bufs=1) as aoutp:
                aout = aoutp.tile([128, NH, S], f32r, tag="aout")
                with tc.tile_pool(name="battn", bufs=2) as battn:
                    mask_sb = battn.tile([128, 4, QW], f32, tag="maskt")
                    nc.sync.dma_start(out=mask_sb[:], in_=maskt[:])

                    blocks = [(h, t) for h in range(NH) for t in range(NQT)]
                    chunks = [(bi, j) for bi, (h, t) in enumerate(blocks)
                              for j in range(4 * (t + 1))]
                    pending = {}   # ci -> pexp tile
                    bstate = {}    # bi -> (po, pd)

                    def emit_score(ci):
                        bi, j = chunks[ci]
                        h, t = blocks[bi]
                        pscore = big()
                        nc.tensor.matmul(
                            pscore[:], kt[:, j * 128:(j + 1) * 128],
                            qt[:, h, t * QW:(t + 1) * QW], start=True, stop=True)
                        if j >= 4 * t:
                            masked = battn.tile([128, QW], f32, tag="msk")
                            nc.vector.tensor_add(masked[:], pscore[:],
                                                 mask_sb[:, j - 4 * t, :])
                            src = masked
                        else:
                            src = pscore
                        pexp = battn.tile([128, QW], f32r, tag="pexp")
                        nc.scalar.activation(pexp[:], src[:], Exp, scale=SCALE)
                        pending[ci] = pexp

                    def emit_consume(ci):
                        bi, j = chunks[ci]
                        h, t = blocks[bi]
                        nkc = 4 * (t + 1)
                        if j == 0:
                            po = ps.tile([128, QW], f32, tag="aux2", bufs=2)
                            pd = ps.tile([1, QW], f32, tag="pd1", bufs=1)
                            bstate[bi] = (po, pd)
                        po, pd = bstate[bi]
                        pexp = pending.pop(ci)
                        nc.tensor.matmul(po[:], vnat[:, j, :], pexp[:],
                                         start=(j == 0), stop=(j == nkc - 1))
                        nc.tensor.matmul(pd[:], onesc[:], pexp[:],
                                         start=(j == 0), stop=(j == nkc - 1))
                        if j == nkc - 1:
                            recip = battn.tile([1, QW], f32, tag="recip")
                            nc.vector.reciprocal(recip[:], pd[:])
                            rbc = battn.tile([128, QW], f32, tag="rbc")
                            nc.gpsimd.partition_broadcast(rbc[:], recip[:])
                            nc.vector.tensor_mul(
                                aout[:, h, t * QW:(t + 1) * QW], po[:], rbc[:])

                    for ci in range(len(chunks) + LOOKAHEAD):
                        if ci < len(chunks):
                            emit_score(ci)
                        if ci >= LOOKAHEAD:
                            emit_consume(ci - LOOKAHEAD)

                # ---- Phase C: output projection (partial) ----
                with tc.tile_pool(name="oproj", bufs=3) as oproj:
                    for n in range(32):
                        wo_t = oproj.tile([128, 4, 128], f32r, tag="wo")
                        nc.sync.dma_start(out=wo_t[:], in_=wo[n])
                        psos = [big(name=f"pso{st}") for st in range(NQT)]
                        for hd in range(4):
                            for st in range(NQT):
                                nc.tensor.matmul(
                                    psos[st][:], wo_t[:, hd, :],
                                    aout[:, hd, st * QW:(st + 1) * QW],
                                    start=(hd == 0), stop=(hd == 3))
                        for st in range(NQT):
                            ob = oproj.tile([128, QW], f32, tag="ob")
                            nc.vector.tensor_copy(ob[:], psos[st][:])
                            nc.sync.dma_start(
                                out=outp[n, :, st * QW:(st + 1) * QW], in_=ob[:])

    nc.compile()
    return nc


# even/odd de-interleave permutation of the head dim
_PERM = np.concatenate([np.arange(0, D, 2), np.arange(1, D, 2)])


def _prep_inputs(x, rotary_emb, attention_mask, Wqkv, bqkv, Wo):
    x = np.asarray(x, dtype=np.float32)
    rotary_emb = np.asarray(rotary_emb, dtype=np.float32)
    attention_mask = np.asarray(attention_mask, dtype=np.float32)
    Wqkv = np.asarray(Wqkv, dtype=np.float32)
    bqkv = np.asarray(bqkv, dtype=np.float32)
    Wo = np.asarray(Wo, dtype=np.float32)

    # x [1,S,NS] -> xa [SC, 128(p=ns%128), NSC, 128(s%128)]
    xa = np.ascontiguousarray(
        x[0].reshape(SC, 128, NSC, 128).transpose(0, 3, 2, 1))

    # rope tables in natural layout [s%128, sc, i]
    cosn = np.ascontiguousarray(
        rotary_emb[:, 0, 0, :, 0].reshape(SC, 128, 64).transpose(1, 0, 2))
    sinn = np.ascontiguousarray(
        rotary_emb[:, 0, 0, :, 1].reshape(SC, 128, 64).transpose(1, 0, 2))

    # diagonal-relative mask tiles [128(k), 4(c), 512(q)], pre-scaled by 1/SCALE
    m = attention_mask[0]
    maskt = np.ascontiguousarray(
        np.stack([m[0:QW, 128 * c:128 * (c + 1)].T for c in range(4)], axis=1)
        * (1.0 / SCALE)).astype(np.float32)

    ident = np.eye(128, dtype=np.float32)
    ones_c = np.ones((128, 1), dtype=np.float32)

    in_maps = []
    for core in range(NCORES):
        g = core // 4
        hg0 = 4 * (core % 4)
        # row indices into Wqkv, column order [k, q0..q3, v]
        rows = [NS + g * D + _PERM]
        for hh in range(NH):
            rows.append((g * HG + hg0 + hh) * D + _PERM)
        rows.append(NS + G * D + g * D + np.arange(D))
        rows = np.concatenate(rows)                      # [768]
        w_core = Wqkv[rows, :]                           # [768, NS]
        # w [128(p=ns%128), NSC, 768]
        w_arr = np.ascontiguousarray(
            w_core.T.reshape(NSC, 128, 768).transpose(1, 0, 2))
        bias_row = bqkv[rows]                            # [768]
        bias_bcast = np.ascontiguousarray(
            np.broadcast_to(bias_row[None, :], (128, 768))).astype(np.float32)
        # wo [32(n), 128(p=hd%128), 4(hd chunk), 128(nn)]
        cols = np.arange((g * HG + hg0) * D, (g * HG + hg0 + NH) * D)
        wo_arr = np.ascontiguousarray(
            Wo[:, cols].reshape(32, 128, 4, 128).transpose(0, 3, 2, 1))
        in_maps.append({
            "xa": xa, "w": w_arr, "wo": wo_arr, "cosn": cosn, "sinn": sinn,
            "maskt": maskt, "bias_b": bias_bcast, "ident_i": ident,
            "ones_c": ones_c,
        })
    return in_maps


def kernel(x, rotary_emb, attention_mask, Wqkv, bqkv, Wo, _trace=False):
    if "nc" not in _CACHE:
        _CACHE["nc"] = _build_program()
    nc = _CACHE["nc"]
    in_maps = _prep_inputs(x, rotary_emb, attention_mask, Wqkv, bqkv, Wo)
    res = run_bass_kernel_spmd(nc, in_maps, list(range(NCORES)), trace=_trace)
    _CACHE["last_result"] = res

    # unshard: sum o_proj partials, transpose back to [1, S, NS]
    out_t = np.zeros((NS, S), dtype=np.float32)
    for core in range(NCORES):
        out_t += res.results[core]["outp"].reshape(NS, S)
    out = np.ascontiguousarray(out_t.T)[None]            # [1, S, NS]

    inv_perm = np.argsort(_PERM)
    k_cache = np.empty((B, S, G, 1, D), dtype=np.float32)
    v_cache = np.empty((B, S, G, 1, D), dtype=np.float32)
    for g, core in ((0, 0), (1, 4)):
        kt_rows = res.results[core]["kc"]                # [128(perm'd d), S]
        k_cache[0, :, g, 0, :] = kt_rows[inv_perm, :].T
        vn = res.results[core]["vc"]                     # [128(s%128), SC, 128]
        v_cache[0, :, g, 0, :] = vn.transpose(1, 0, 2).reshape(S, D)
    return out, k_cache, v_cache


# revision 4
# speedup vs baseline: 1.0949x; 1.0221x over previous
"""ChatGLM2 attention block on 8 Trainium2 NeuronCores.

Tensor-parallel across heads: core c computes KV-group c//4 and Q-heads
4*(c%4)..4*(c%4)+3, plus the matching row-slice of the output projection.
Partial o_proj outputs are summed on the host (row-parallel unshard).
All matmuls run in float32r (reduced-precision fp32, ~1e-4 rel err).
"""

import numpy as np

import concourse.bass as bass
import concourse.mybir as mybir
from concourse import bacc
from concourse.tile import TileContext
from concourse.bass_utils import run_bass_kernel_spmd

# problem shape (hardcoded per contract)
B, S, NS, H, G, D = 1, 2048, 4096, 32, 2, 128
HG = H // G          # 16 heads per group
NH = 4               # q heads per core
NCORES = 8
NSC = NS // 128      # 32 contraction chunks
SC = S // 128        # 16 sequence chunks
NQT = 4              # q tiles of 512
QW = 512             # q tile width
SCALE = 1.0 / float(np.sqrt(D))
LOOKAHEAD = 4        # attention pscore pipeline depth

f32 = mybir.dt.float32
f32r = mybir.dt.float32r

_CACHE = {}


def _build_program():
    nc = bacc.Bacc("TRN2", target_bir_lowering=False, debug=False,
                   num_devices=NCORES)

    xa = nc.declare_dram_parameter("xa", [SC, 128, NSC, 128], f32r, isOutput=False)
    w = nc.declare_dram_parameter("w", [128, NSC, 768], f32r, isOutput=False)
    wo = nc.declare_dram_parameter("wo", [32, 128, 4, 128], f32r, isOutput=False)
    cosn = nc.declare_dram_parameter("cosn", [128, SC, 64], f32, isOutput=False)
    sinn = nc.declare_dram_parameter("sinn", [128, SC, 64], f32, isOutput=False)
    maskt = nc.declare_dram_parameter("maskt", [128, 4, QW], f32, isOutput=False)
    bias_b = nc.declare_dram_parameter("bias_b", [128, 768], f32, isOutput=False)
    ident_i = nc.declare_dram_parameter("ident_i", [128, 128], f32r, isOutput=False)
    ones_c = nc.declare_dram_parameter("ones_c", [128, 1], f32r, isOutput=False)
    ones_r = nc.declare_dram_parameter("ones_r", [1, 128], f32r, isOutput=False)

    outp = nc.declare_dram_parameter("outp", [32, 128, S], f32, isOutput=True)
    kc = nc.declare_dram_parameter("kc", [128, S], f32r, isOutput=True)
    vc = nc.declare_dram_parameter("vc", [128, SC, 128], f32r, isOutput=True)

    Copy = mybir.ActivationFunctionType.Copy
    Exp = mybir.ActivationFunctionType.Exp

    with TileContext(nc) as tc:
        with (
            tc.tile_pool(name="persist", bufs=1) as persist,
            tc.tile_pool(name="ps", bufs=1, space="PSUM") as ps,
        ):
            def big(name="big"):
                return ps.tile([128, 512], f32, tag="big5", bufs=5, name=name)

            qt = persist.tile([128, NH, S], f32r, tag="qt")
            kt = persist.tile([128, S], f32r, tag="kt")
            vnat = persist.tile([128, SC, 128], f32r, tag="vnat")
            cos_sb = persist.tile([128, SC, 64], f32, tag="cos")
            sin_sb = persist.tile([128, SC, 64], f32, tag="sin")
            bias_sb = persist.tile([128, 768], f32, tag="bias")
            ident = persist.tile([128, 128], f32r, tag="ident")
            onesc = persist.tile([128, 1], f32r, tag="onesc")
            onesr = persist.tile([1, 128], f32r, tag="onesr")
            nc.sync.dma_start(out=cos_sb[:], in_=cosn[:])
            nc.sync.dma_start(out=sin_sb[:], in_=sinn[:])
            nc.sync.dma_start(out=bias_sb[:], in_=bias_b[:])
            nc.sync.dma_start(out=ident[:], in_=ident_i[:])
            nc.sync.dma_start(out=onesc[:], in_=ones_c[:])
            nc.sync.dma_start(out=onesr[:], in_=ones_r[:])

            # ---- Phase A: QKV projection + bias + RoPE + Q/K transpose ----
            with (
                tc.tile_pool(name="wpool", bufs=1) as wpool,
                tc.tile_pool(name="xsp", bufs=2) as xsp,
                tc.tile_pool(name="stage", bufs=2) as stage,
            ):
                xs_tiles = {}

                def prefetch_xs(sc):
                    if sc < SC:
                        xs = xsp.tile([128, NSC, 128], f32r, tag="xs",
                                      name=f"xs{sc}")
                        nc.sync.dma_start(out=xs[:], in_=xa[sc])
                        xs_tiles[sc] = xs

                prefetch_xs(0)
                w_parts = []
                for i in range(8):
                    wp = wpool.tile([128, 4, 768], f32r, tag=f"w{i}")
                    nc.sync.dma_start(out=wp[:], in_=w[:, i * 4:(i + 1) * 4, :])
                    w_parts.append(wp)

                def emit_proj(sc):
                    prefetch_xs(sc + 1)
                    xs = xs_tiles.pop(sc)
                    psA = big()
                    psB = ps.tile([128, 256], f32, tag="aux2", bufs=2)
                    for ns in range(NSC):
                        wv = w_parts[ns // 4][:, ns % 4, :]
                        nc.tensor.matmul(psA[:], xs[:, ns, :], wv[:, 0:512],
                                         start=(ns == 0), stop=(ns == NSC - 1))
                        nc.tensor.matmul(psB[:], xs[:, ns, :], wv[:, 512:768],
                                         start=(ns == 0), stop=(ns == NSC - 1))
                    return psA, psB

                def emit_tail(sc, psA, psB):
                    # bias add -> natural fp32 staging (rope cols 0:640)
                    nat = stage.tile([128, 640], f32, tag="nat")
                    nc.vector.tensor_add(nat[:, 0:512], psA[:], bias_sb[:, 0:512])
                    nc.vector.tensor_add(nat[:, 512:640], psB[:, 0:128],
                                         bias_sb[:, 512:640])
                    nc.vector.tensor_add(vnat[:, sc, :], psB[:, 128:256],
                                         bias_sb[:, 640:768])
                    natv = nat[:].rearrange("p (h t d) -> p h t d", h=5, t=2)
                    even = natv[:, :, 0, :]
                    odd = natv[:, :, 1, :]
                    cb = cos_sb[:, sc:sc + 1, :].broadcast_to([128, 5, 64])
                    sb_ = sin_sb[:, sc:sc + 1, :].broadcast_to([128, 5, 64])
                    rt = stage.tile([128, 5, 2, 64], f32r, tag="rt")
                    t1 = stage.tile([128, 5, 64], f32, tag="t1")
                    t2 = stage.tile([128, 5, 64], f32, tag="t2")
                    nc.vector.tensor_mul(t1[:], even, cb)
                    nc.vector.tensor_mul(t2[:], odd, sb_)
                    nc.vector.tensor_sub(rt[:, :, 0, :], t1[:], t2[:])
                    t3 = stage.tile([128, 5, 64], f32, tag="t1")
                    t4 = stage.tile([128, 5, 64], f32, tag="t2")
                    nc.vector.tensor_mul(t3[:], even, sb_)
                    nc.vector.tensor_mul(t4[:], odd, cb)
                    nc.vector.tensor_add(rt[:, :, 1, :], t3[:], t4[:])
                    for r in range(5):
                        pt = ps.tile([128, 128], f32r, tag="big5", bufs=5)
                        nc.tensor.transpose(
                            pt[:], rt[:, r, :, :].rearrange("p t d -> p (t d)"),
                            ident[:])
                        dst = kt[:, sc * 128:(sc + 1) * 128] if r == 0 else \
                            qt[:, r - 1, sc * 128:(sc + 1) * 128]
                        nc.scalar.activation(dst, pt[:], Copy)

                # software pipeline: proj(sc) ahead of tail(sc-1)
                prev = None
                for sc in range(SC):
                    cur = emit_proj(sc)
                    if prev is not None:
                        emit_tail(sc - 1, *prev)
                    prev = cur
                emit_tail(SC - 1, *prev)

            # kv cache outputs
            nc.gpsimd.dma_start(out=kc[:], in_=kt[:])
            nc.gpsimd.dma_start(out=vc[:], in_=vnat[:])

            # ---- Phase B: attention ----
            with tc.tile_pool(name="aout", bufs=1) as aoutp:
                aout = aoutp.tile([128, NH, S], f32r, tag="aout")
                with tc.tile_pool(name="battn", bufs=2) as battn:
                    mask_sb = battn.tile([128, 4, QW], f32, tag="maskt")
                    nc.sync.dma_start(out=mask_sb[:], in_=maskt[:])

                    blocks = [(h, t) for h in range(NH) for t in range(NQT)]
                    chunks = [(bi, j) for bi, (h, t) in enumerate(blocks)
                              for j in range(4 * (t + 1))]
                    pending = {}   # ci -> pexp tile
                    bstate = {}    # bi -> (po, pd)

                    def emit_score(ci):
                        bi, j = chunks[ci]
                        h, t = blocks[bi]
                        pscore = big()
                        nc.tensor.matmul(
                            pscore[:], kt[:, j * 128:(j + 1) * 128],
                            qt[:, h, t * QW:(t + 1) * QW], start=True, stop=True)
                        if j >= 4 * t:
                            masked = battn.tile([128, QW], f32, tag="msk")
                            nc.vector.tensor_add(masked[:], pscore[:],
                                                 mask_sb[:, j - 4 * t, :])
                            src = masked
                        else:
                            src = pscore
                        pexp = battn.tile([128, QW], f32r, tag="pexp")
                        nc.scalar.activation(pexp[:], src[:], Exp, scale=SCALE)
                        pending[ci] = pexp

                    def emit_consume(ci):
                        bi, j = chunks[ci]
                        h, t = blocks[bi]
                        nkc = 4 * (t + 1)
                        if j == 0:
                            po = ps.tile([128, QW], f32, tag="aux2", bufs=2)
                            pd = ps.tile([1, QW], f32, tag="pd1", bufs=1)
                            bstate[bi] = (po, pd)
                        po, pd = bstate[bi]
                        pexp = pending.pop(ci)
                        nc.tensor.matmul(po[:], vnat[:, j, :], pexp[:],
                                         start=(j == 0), stop=(j == nkc - 1))
                        nc.tensor.matmul(pd[:], onesc[:], pexp[:],
                                         start=(j == 0), stop=(j == nkc - 1))
                        if j == nkc - 1:
                            recip = battn.tile([1, QW], f32r, tag="recip")
                            with nc.allow_low_precision(reason="f32r storage"):
                                nc.vector.reciprocal(recip[:], pd[:])
                            pb = big(name="pb")
                            nc.tensor.matmul(pb[:], onesr[:], recip[:],
                                             start=True, stop=True)
                            pbs = battn.tile([128, QW], f32, tag="pbs")
                            nc.scalar.activation(pbs[:], pb[:], Copy)
                            nc.vector.tensor_mul(
                                aout[:, h, t * QW:(t + 1) * QW], po[:], pbs[:])

                    for ci in range(len(chunks) + LOOKAHEAD):
                        if ci < len(chunks):
                            emit_score(ci)
                        if ci >= LOOKAHEAD:
                            emit_consume(ci - LOOKAHEAD)

                # ---- Phase C: output projection (partial) ----
                with tc.tile_pool(name="oproj", bufs=3) as oproj:
                    for n in range(32):
                        wo_t = oproj.tile([128, 4, 128], f32r, tag="wo")
                        nc.gpsimd.dma_start(out=wo_t[:], in_=wo[n])
                        psos = [big(name=f"pso{st}") for st in range(NQT)]
                        for hd in range(4):
                            for st in range(NQT):
                                nc.tensor.matmul(
                                    psos[st][:], wo_t[:, hd, :],
                                    aout[:, hd, st * QW:(st + 1) * QW],
                                    start=(hd == 0), stop=(hd == 3))
                        for st in range(NQT):
                            ob = oproj.tile([128, QW], f32, tag="ob", bufs=4)
                            nc.scalar.activation(ob[:], psos[st][:], Copy)
                            nc.sync.dma_start(
                                out=outp[n, :, st * QW:(st + 1) * QW], in_=ob[:])

    nc.compile()
    return nc


# even/odd de-interleave permutation of the head dim
_PERM = np.concatenate([np.arange(0, D, 2), np.arange(1, D, 2)])


def _prep_inputs(x, rotary_emb, attention_mask, Wqkv, bqkv, Wo):
    x = np.asarray(x, dtype=np.float32)
    rotary_emb = np.asarray(rotary_emb, dtype=np.float32)
    attention_mask = np.asarray(attention_mask, dtype=np.float32)
    Wqkv = np.asarray(Wqkv, dtype=np.float32)
    bqkv = np.asarray(bqkv, dtype=np.float32)
    Wo = np.asarray(Wo, dtype=np.float32)

    # x [1,S,NS] -> xa [SC, 128(p=ns%128), NSC, 128(s%128)]
    xa = np.ascontiguousarray(
        x[0].reshape(SC, 128, NSC, 128).transpose(0, 3, 2, 1))

    # rope tables in natural layout [s%128, sc, i]
    cosn = np.ascontiguousarray(
        rotary_emb[:, 0, 0, :, 0].reshape(SC, 128, 64).transpose(1, 0, 2))
    sinn = np.ascontiguousarray(
        rotary_emb[:, 0, 0, :, 1].reshape(SC, 128, 64).transpose(1, 0, 2))

    # diagonal-relative mask tiles [128(k), 4(c), 512(q)], pre-scaled by 1/SCALE
    m = attention_mask[0]
    maskt = np.ascontiguousarray(
        np.stack([m[0:QW, 128 * c:128 * (c + 1)].T for c in range(4)], axis=1)
        * (1.0 / SCALE)).astype(np.float32)

    ident = np.eye(128, dtype=np.float32)
    ones_c = np.ones((128, 1), dtype=np.float32)
    ones_r = np.ones((1, 128), dtype=np.float32)

    in_maps = []
    for core in range(NCORES):
        g = core // 4
        hg0 = 4 * (core % 4)
        # row indices into Wqkv, column order [k, q0..q3, v]
        rows = [NS + g * D + _PERM]
        for hh in range(NH):
            rows.append((g * HG + hg0 + hh) * D + _PERM)
        rows.append(NS + G * D + g * D + np.arange(D))
        rows = np.concatenate(rows)                      # [768]
        w_core = Wqkv[rows, :]                           # [768, NS]
        # w [128(p=ns%128), NSC, 768]
        w_arr = np.ascontiguousarray(
            w_core.T.reshape(NSC, 128, 768).transpose(1, 0, 2))
        bias_row = bqkv[rows]                            # [768]
        bias_bcast = np.ascontiguousarray(
            np.broadcast_to(bias_row[None, :], (128, 768))).astype(np.float32)
        # wo [32(n), 128(p=hd%128), 4(hd chunk), 128(nn)]
        cols = np.arange((g * HG + hg0) * D, (g * HG + hg0 + NH) * D)
        wo_arr = np.ascontiguousarray(
            Wo[:, cols].reshape(32, 128, 4, 128).transpose(0, 3, 2, 1))
        in_maps.append({
            "xa": xa, "w": w_arr, "wo": wo_arr, "cosn": cosn, "sinn": sinn,
            "maskt": maskt, "bias_b": bias_bcast, "ident_i": ident,
            "ones_c": ones_c, "ones_r": ones_r,
        })
    return in_maps


def kernel(x, rotary_emb, attention_mask, Wqkv, bqkv, Wo, _trace=False):
    if "nc" not in _CACHE:
        _CACHE["nc"] = _build_program()
    nc = _CACHE["nc"]
    in_maps = _prep_inputs(x, rotary_emb, attention_mask, Wqkv, bqkv, Wo)
    res = run_bass_kernel_spmd(nc, in_maps, list(range(NCORES)), trace=_trace)
    _CACHE["last_result"] = res

    # unshard: sum o_proj partials, transpose back to [1, S, NS]
    out_t = np.zeros((NS, S), dtype=np.float32)
    for core in range(NCORES):
        out_t += res.results[core]["outp"].reshape(NS, S)
    out = np.ascontiguousarray(out_t.T)[None]            # [1, S, NS]

    inv_perm = np.argsort(_PERM)
    k_cache = np.empty((B, S, G, 1, D), dtype=np.float32)
    v_cache = np.empty((B, S, G, 1, D), dtype=np.float32)
    for g, core in ((0, 0), (1, 4)):
        kt_rows = res.results[core]["kc"]                # [128(perm'd d), S]
        k_cache[0, :, g, 0, :] = kt_rows[inv_perm, :].T
        vn = res.results[core]["vc"]                     # [128(s%128), SC, 128]
        v_cache[0, :, g, 0, :] = vn.transpose(1, 0, 2).reshape(S, D)
    return out, k_cache, v_cache


# revision 6
# speedup vs baseline: 1.1301x; 1.0322x over previous
"""ChatGLM2 attention block on 8 Trainium2 NeuronCores.

Tensor-parallel across heads: core c computes KV-group c//4 and Q-heads
4*(c%4)..4*(c%4)+3, plus the matching row-slice of the output projection.
Partial o_proj outputs are summed on the host (row-parallel unshard).
All matmuls run in float32r (reduced-precision fp32, ~1e-4 rel err).
"""

import numpy as np

import concourse.bass as bass
import concourse.mybir as mybir
from concourse import bacc
from concourse.tile import TileContext
from concourse.bass_utils import run_bass_kernel_spmd

# problem shape (hardcoded per contract)
B, S, NS, H, G, D = 1, 2048, 4096, 32, 2, 128
HG = H // G          # 16 heads per group
NH = 4               # q heads per core
NCORES = 8
NSC = NS // 128      # 32 contraction chunks
SC = S // 128        # 16 sequence chunks
NQT = 4              # q tiles of 512
QW = 512             # q tile width
SCALE = 1.0 / float(np.sqrt(D))
LOOKAHEAD = 3        # attention pscore pipeline depth

f32 = mybir.dt.float32
f32r = mybir.dt.float32r

_CACHE = {}


def _build_program():
    nc = bacc.Bacc("TRN2", target_bir_lowering=False, debug=False,
                   num_devices=NCORES)

    xa = nc.declare_dram_parameter("xa", [SC, 128, NSC, 128], f32r, isOutput=False)
    w = nc.declare_dram_parameter("w", [128, NSC, 768], f32r, isOutput=False)
    wo = nc.declare_dram_parameter("wo", [32, 128, 4, 128], f32r, isOutput=False)
    cosn = nc.declare_dram_parameter("cosn", [128, SC, 64], f32, isOutput=False)
    sinn = nc.declare_dram_parameter("sinn", [128, SC, 64], f32, isOutput=False)
    maskt = nc.declare_dram_parameter("maskt", [128, 4, QW], f32, isOutput=False)
    bias_b = nc.declare_dram_parameter("bias_b", [128, 768], f32, isOutput=False)
    ident_i = nc.declare_dram_parameter("ident_i", [128, 128], f32r, isOutput=False)
    ones_c = nc.declare_dram_parameter("ones_c", [128, 1], f32r, isOutput=False)
    ones_r = nc.declare_dram_parameter("ones_r", [1, 128], f32r, isOutput=False)

    outp = nc.declare_dram_parameter("outp", [32, 128, S], f32, isOutput=True)
    kc = nc.declare_dram_parameter("kc", [128, S], f32r, isOutput=True)
    vc = nc.declare_dram_parameter("vc", [128, SC, 128], f32r, isOutput=True)

    Copy = mybir.ActivationFunctionType.Copy
    Exp = mybir.ActivationFunctionType.Exp

    with TileContext(nc) as tc:
        with (
            tc.tile_pool(name="persist", bufs=1) as persist,
            tc.tile_pool(name="ps", bufs=1, space="PSUM") as ps,
        ):
            def big(name="big"):
                return ps.tile([128, 512], f32, tag="big5", bufs=4, name=name)

            qt = persist.tile([128, NH, S], f32r, tag="qt")
            kt = persist.tile([128, S], f32r, tag="kt")
            vnat = persist.tile([128, SC, 128], f32r, tag="vnat")
            cos_sb = persist.tile([128, SC, 64], f32, tag="cos")
            sin_sb = persist.tile([128, SC, 64], f32, tag="sin")
            bias_sb = persist.tile([128, 768], f32, tag="bias")
            ident = persist.tile([128, 128], f32r, tag="ident")
            onesc = persist.tile([128, 1], f32r, tag="onesc")
            onesr = persist.tile([1, 128], f32r, tag="onesr")
            nc.sync.dma_start(out=cos_sb[:], in_=cosn[:])
            nc.sync.dma_start(out=sin_sb[:], in_=sinn[:])
            nc.sync.dma_start(out=bias_sb[:], in_=bias_b[:])
            nc.sync.dma_start(out=ident[:], in_=ident_i[:])
            nc.sync.dma_start(out=onesc[:], in_=ones_c[:])
            nc.sync.dma_start(out=onesr[:], in_=ones_r[:])

            # ---- Phase A: QKV projection + bias + RoPE + Q/K transpose ----
            with (
                tc.tile_pool(name="wpool", bufs=1) as wpool,
                tc.tile_pool(name="xsp", bufs=2) as xsp,
                tc.tile_pool(name="stage", bufs=2) as stage,
            ):
                xs_tiles = {}

                def prefetch_xs(sc):
                    if sc < SC:
                        xs = xsp.tile([128, NSC, 128], f32r, tag="xs",
                                      name=f"xs{sc}")
                        nc.sync.dma_start(out=xs[:], in_=xa[sc])
                        xs_tiles[sc] = xs

                prefetch_xs(0)
                w_parts = []
                for i in range(8):
                    wp = wpool.tile([128, 4, 768], f32r, tag=f"w{i}")
                    nc.sync.dma_start(out=wp[:], in_=w[:, i * 4:(i + 1) * 4, :])
                    w_parts.append(wp)
                    if i == 1:
                        prefetch_xs(1)

                def emit_proj(sc):
                    if sc + 1 != 1:
                        prefetch_xs(sc + 1)
                    xs = xs_tiles.pop(sc)
                    psA = big()
                    psB = ps.tile([128, 256], f32, tag="aux2", bufs=3)
                    for ns in range(NSC):
                        wv = w_parts[ns // 4][:, ns % 4, :]
                        nc.tensor.matmul(psA[:], xs[:, ns, :], wv[:, 0:512],
                                         start=(ns == 0), stop=(ns == NSC - 1))
                        nc.tensor.matmul(psB[:], xs[:, ns, :], wv[:, 512:768],
                                         start=(ns == 0), stop=(ns == NSC - 1))
                    return psA, psB

                def emit_tail(sc, psA, psB):
                    # bias add -> natural fp32 staging (rope cols 0:640)
                    nat = stage.tile([128, 640], f32, tag="nat")
                    nc.vector.tensor_add(nat[:, 0:512], psA[:], bias_sb[:, 0:512])
                    nc.vector.tensor_add(nat[:, 512:640], psB[:, 0:128],
                                         bias_sb[:, 512:640])
                    nc.vector.tensor_add(vnat[:, sc, :], psB[:, 128:256],
                                         bias_sb[:, 640:768])
                    natv = nat[:].rearrange("p (h t d) -> p h t d", h=5, t=2)
                    even = natv[:, :, 0, :]
                    odd = natv[:, :, 1, :]
                    cb = cos_sb[:, sc:sc + 1, :].broadcast_to([128, 5, 64])
                    sb_ = sin_sb[:, sc:sc + 1, :].broadcast_to([128, 5, 64])
                    rt = stage.tile([128, 5, 2, 64], f32r, tag="rt")
                    t1 = stage.tile([128, 5, 64], f32, tag="t1")
                    t2 = stage.tile([128, 5, 64], f32, tag="t2")
                    nc.vector.tensor_mul(t1[:], even, cb)
                    nc.vector.tensor_mul(t2[:], odd, sb_)
                    nc.vector.tensor_sub(rt[:, :, 0, :], t1[:], t2[:])
                    t3 = stage.tile([128, 5, 64], f32, tag="t1")
                    t4 = stage.tile([128, 5, 64], f32, tag="t2")
                    nc.vector.tensor_mul(t3[:], even, sb_)
                    nc.vector.tensor_mul(t4[:], odd, cb)
                    nc.vector.tensor_add(rt[:, :, 1, :], t3[:], t4[:])
                    for r in range(5):
                        pt = ps.tile([128, 128], f32r, tag="big5", bufs=4)
                        nc.tensor.transpose(
                            pt[:], rt[:, r, :, :].rearrange("p t d -> p (t d)"),
                            ident[:])
                        dst = kt[:, sc * 128:(sc + 1) * 128] if r == 0 else \
                            qt[:, r - 1, sc * 128:(sc + 1) * 128]
                        nc.scalar.activation(dst, pt[:], Copy)

                # software pipeline: proj(sc) ahead of tail(sc-1)
                prev = None
                for sc in range(SC):
                    cur = emit_proj(sc)
                    if prev is not None:
                        emit_tail(sc - 1, *prev)
                    prev = cur
                emit_tail(SC - 1, *prev)

            # kv cache outputs
            nc.sync.dma_start(out=kc[:], in_=kt[:])
            nc.sync.dma_start(out=vc[:], in_=vnat[:])

            # ---- Phase B: attention ----
            with tc.tile_pool(name="aout", bufs=1) as aoutp:
                aout = aoutp.tile([128, NH, S], f32r, tag="aout")
                wo_all = aoutp.tile([128, 32, 4, 128], f32r, tag="wo_all")
                nc.sync.dma_start(
                    out=wo_all[:], in_=wo[:].rearrange("n p h d -> p n h d"))
                with tc.tile_pool(name="battn", bufs=2) as battn:
                    mask_sb = battn.tile([128, 4, QW], f32, tag="maskt")
                    nc.sync.dma_start(out=mask_sb[:], in_=maskt[:])

                    blocks = [(h, t) for h in range(NH) for t in range(NQT)]
                    chunks = [(bi, j) for bi, (h, t) in enumerate(blocks)
                              for j in range(4 * (t + 1))]
                    pending = {}   # ci -> pexp tile
                    bstate = {}    # bi -> (po, pd)

                    def emit_score(ci):
                        bi, j = chunks[ci]
                        h, t = blocks[bi]
                        pscore = big()
                        nc.tensor.matmul(
                            pscore[:], kt[:, j * 128:(j + 1) * 128],
                            qt[:, h, t * QW:(t + 1) * QW], start=True, stop=True)
                        if j >= 4 * t:
                            masked = battn.tile([128, QW], f32, tag="msk")
                            nc.vector.tensor_add(masked[:], pscore[:],
                                                 mask_sb[:, j - 4 * t, :])
                            src = masked
                        else:
                            src = pscore
                        pexp = battn.tile([128, QW], f32r, tag="pexp")
                        nc.scalar.activation(pexp[:], src[:], Exp, scale=SCALE)
                        pending[ci] = pexp

                    def emit_consume(ci):
                        bi, j = chunks[ci]
                        h, t = blocks[bi]
                        nkc = 4 * (t + 1)
                        if j == 0:
                            po = ps.tile([128, QW], f32, tag="aux2", bufs=3)
                            pd = ps.tile([1, QW], f32, tag="pd1", bufs=1)
                            bstate[bi] = (po, pd)
                        po, pd = bstate[bi]
                        pexp = pending.pop(ci)
                        nc.tensor.matmul(po[:], vnat[:, j, :], pexp[:],
                                         start=(j == 0), stop=(j == nkc - 1))
                        nc.tensor.matmul(pd[:], onesc[:], pexp[:],
                                         start=(j == 0), stop=(j == nkc - 1))
                        if j == nkc - 1:
                            recip = battn.tile([1, QW], f32r, tag="recip")
                            with nc.allow_low_precision(reason="f32r storage"):
                                nc.vector.reciprocal(recip[:], pd[:])
                            pb = big(name="pb")
                            nc.tensor.matmul(pb[:], onesr[:], recip[:],
                                             start=True, stop=True)
                            pbs = battn.tile([128, QW], f32, tag="pbs")
                            nc.vector.tensor_copy(pbs[:], pb[:])
                            nc.vector.tensor_mul(
                                aout[:, h, t * QW:(t + 1) * QW], po[:], pbs[:])

                    for ci in range(len(chunks) + LOOKAHEAD):
                        if ci < len(chunks):
                            emit_score(ci)
                        if ci >= LOOKAHEAD:
                            emit_consume(ci - LOOKAHEAD)

                # ---- Phase C: output projection (partial) ----
                with tc.tile_pool(name="oproj", bufs=3) as oproj:
                    for n in range(32):
                        psos = [big(name=f"pso{st}") for st in range(NQT)]
                        for hd in range(4):
                            for st in range(NQT):
                                nc.tensor.matmul(
                                    psos[st][:], wo_all[:, n, hd, :],
                                    aout[:, hd, st * QW:(st + 1) * QW],
                                    start=(hd == 0), stop=(hd == 3))
                                if hd == 3:
                                    ob = oproj.tile([128, QW], f32, tag="ob",
                                                    bufs=4, name=f"ob{st}")
                                    nc.scalar.activation(ob[:], psos[st][:], Copy)
                                    nc.sync.dma_start(
                                        out=outp[n, :, st * QW:(st + 1) * QW],
                                        in_=ob[:])

    nc.compile()
    return nc


# even/odd de-interleave permutation of the head dim
_PERM = np.concatenate([np.arange(0, D, 2), np.arange(1, D, 2)])


def _prep_inputs(x, rotary_emb, attention_mask, Wqkv, bqkv, Wo):
    x = np.asarray(x, dtype=np.float32)
    rotary_emb = np.asarray(rotary_emb, dtype=np.float32)
    attention_mask = np.asarray(attention_mask, dtype=np.float32)
    Wqkv = np.asarray(Wqkv, dtype=np.float32)
    bqkv = np.asarray(bqkv, dtype=np.float32)
    Wo = np.asarray(Wo, dtype=np.float32)

    # x [1,S,NS] -> xa [SC, 128(p=ns%128), NSC, 128(s%128)]
    xa = np.ascontiguousarray(
        x[0].reshape(SC, 128, NSC, 128).transpose(0, 3, 2, 1))

    # rope tables in natural layout [s%128, sc, i]
    cosn = np.ascontiguousarray(
        rotary_emb[:, 0, 0, :, 0].reshape(SC, 128, 64).transpose(1, 0, 2))
    sinn = np.ascontiguousarray(
        rotary_emb[:, 0, 0, :, 1].reshape(SC, 128, 64).transpose(1, 0, 2))

    # diagonal-relative mask tiles [128(k), 4(c), 512(q)], pre-scaled by 1/SCALE
    m = attention_mask[0]
    maskt = np.ascontiguousarray(
        np.stack([m[0:QW, 128 * c:128 * (c + 1)].T for c in range(4)], axis=1)
        * (1.0 / SCALE)).astype(np.float32)

    ident = np.eye(128, dtype=np.float32)
    ones_c = np.ones((128, 1), dtype=np.float32)
    ones_r = np.ones((1, 128), dtype=np.float32)

    in_maps = []
    for core in range(NCORES):
        g = core // 4
        hg0 = 4 * (core % 4)
        # row indices into Wqkv, column order [k, q0..q3, v]
        rows = [NS + g * D + _PERM]
        for hh in range(NH):
            rows.append((g * HG + hg0 + hh) * D + _PERM)
        rows.append(NS + G * D + g * D + np.arange(D))
        rows = np.concatenate(rows)                      # [768]
        w_core = Wqkv[rows, :]                           # [768, NS]
        # w [128(p=ns%128), NSC, 768]
        w_arr = np.ascontiguousarray(
            w_core.T.reshape(NSC, 128, 768).transpose(1, 0, 2))
        bias_row = bqkv[rows]                            # [768]
        bias_bcast = np.ascontiguousarray(
            np.broadcast_to(bias_row[None, :], (128, 768))).astype(np.float32)
        # wo [32(n), 128(p=hd%128), 4(hd chunk), 128(nn)]
        cols = np.arange((g * HG + hg0) * D, (g * HG + hg0 + NH) * D)
        wo_arr = np.ascontiguousarray(
            Wo[:, cols].reshape(32, 128, 4, 128).transpose(0, 3, 2, 1))
        in_maps.append({
            "xa": xa, "w": w_arr, "wo": wo_arr, "cosn": cosn, "sinn": sinn,
            "maskt": maskt, "bias_b": bias_bcast, "ident_i": ident,
            "ones_c": ones_c, "ones_r": ones_r,
        })
    return in_maps


def kernel(x, rotary_emb, attention_mask, Wqkv, bqkv, Wo, _trace=False):
    if "nc" not in _CACHE:
        _CACHE["nc"] = _build_program()
    nc = _CACHE["nc"]
    in_maps = _prep_inputs(x, rotary_emb, attention_mask, Wqkv, bqkv, Wo)
    res = run_bass_kernel_spmd(nc, in_maps, list(range(NCORES)), trace=_trace)
    _CACHE["last_result"] = res

    # unshard: sum o_proj partials, transpose back to [1, S, NS]
    out_t = np.zeros((NS, S), dtype=np.float32)
    for core in range(NCORES):
        out_t += res.results[core]["outp"].reshape(NS, S)
    out = np.ascontiguousarray(out_t.T)[None]            # [1, S, NS]

    inv_perm = np.argsort(_PERM)
    k_cache = np.empty((B, S, G, 1, D), dtype=np.float32)
    v_cache = np.empty((B, S, G, 1, D), dtype=np.float32)
    for g, core in ((0, 0), (1, 4)):
        kt_rows = res.results[core]["kc"]                # [128(perm'd d), S]
        k_cache[0, :, g, 0, :] = kt_rows[inv_perm, :].T
        vn = res.results[core]["vc"]                     # [128(s%128), SC, 128]
        v_cache[0, :, g, 0, :] = vn.transpose(1, 0, 2).reshape(S, D)
    return out, k_cache, v_cache
